# revision 45
# baseline (speedup 1.0000x reference)
"""Multi-head attention (B=8, S=1024, E=768, H=12, D=64) on 8 TRN2 NeuronCores.

Sharding: data-parallel over batch. Core i computes batch element i end to end;
weights are replicated. No collectives.

Per-core dataflow (matmuls in bf16; f32 PSUM accumulation):
  1. x [S,E] -> PE-transpose -> xT [E,S] (bf16)
  2. w_qkv DMA'd f32 into staging, cast to bf16 on GPSIMD (idle engine)
  3. QT/KT = w_qkv.T @ xT; V -> V_pad [S, H*(D+1)] bf16 with a ones column
  4. attention as a flat schedule of 96 slots (12 head-pair-phases x 8 key
     chunks). Slot g: scores (2 concurrent K=64 matmuls via tile_position)
     + one [128,1024] exp on ACT; PV matmuls run LAGGED by a full phase (8
     slots) so the PE never head-of-line blocks on late V DMA and the PV
     psum hand-off carries no bubble. Each phase's PV result is copied to
     SBUF immediately (freeing the psum bank), then normalized off the
     critical path (denominator broadcast matmul + fast reciprocal + mul).
  5. y = attnT.T @ w_out + b_out (bf16); the ej=0..3 contraction chunks run
     as fillers inside late attention slots, only ej=4..5 trail the loop.

Projection work (QKT/V chunks) is scheduled into specific slots as PE
filler, just-in-time with DMA arrival: x, QK pair 0, V, QK pairs 1-3, 4-5.

PSUM budget (8 banks): scores 2 x 2 + pv 2 x 1 + mm 2 x 1.
"""

import numpy as np

import concourse.bass as bass
import concourse.bacc as bacc
import concourse.tile as tile
from concourse import mybir
from concourse.bass_utils import run_bass_kernel_spmd
from concourse.bass_interp import get_hw_module
from concourse.masks import make_identity

F32 = mybir.dt.float32
F32R = mybir.dt.float32r
BF16 = mybir.dt.bfloat16
U32 = mybir.dt.uint32

B, S, E = 8, 1024, 768
H, D = 12, 64
F = 3 * E                  # 2304
NCORES = 8
NPAIR = H // 2             # 6 head pairs
NKC = S // 128             # 8 key chunks
NST = S // 128             # 8 sequence tiles
NE = E // 128              # 6 embedding chunks
DP = D + 1                 # 65: head dim + ones column
NPH = 2 * NPAIR            # 12 phases (pair, q-half)
PVLAG = 8                  # PV trails scores by one full phase
NSLOT = NPH * NKC          # 96 scored slots
NDRAIN = 20                # trailing slots to flush pv + norm tails


def _build():
    nc = bacc.Bacc("TRN2", target_bir_lowering=False, debug=False,
                   num_devices=NCORES)

    x_d = nc.dram_tensor("x", [S, E], F32, kind="ExternalInput").ap()
    wqkv_d = nc.dram_tensor("w_qkv", [E, F], F32, kind="ExternalInput").ap()
    wout_d = nc.dram_tensor("w_out", [E, E], F32, kind="ExternalInput").ap()
    bout_d = nc.dram_tensor("b_out", [E], F32, kind="ExternalInput").ap()
    y_d = nc.dram_tensor("y", [S, E], F32, kind="ExternalOutput").ap()

    with tile.TileContext(nc) as tc:
        _emit(nc, tc, x_d, wqkv_d, wout_d, bout_d, y_d)

    nc.compile()
    nc.m = get_hw_module(nc.m)
    return nc


def _emit(nc, tc, x_d, wqkv_d, wout_d, bout_d, y_d):
    from contextlib import ExitStack
    ctx = ExitStack()
    with ctx:
        singles = ctx.enter_context(tc.tile_pool(name="singles", bufs=1))
        sb = ctx.enter_context(tc.tile_pool(name="sb", bufs=1))
        ps = ctx.enter_context(tc.tile_pool(name="ps", bufs=1, space="PSUM"))
        expst_pool = ctx.enter_context(tc.tile_pool(name="expst", bufs=10))
        bcast_pool = ctx.enter_context(tc.tile_pool(name="bcast", bufs=2))
        pvsb_pool = ctx.enter_context(tc.tile_pool(name="pvsb", bufs=4))
        rc_pool = ctx.enter_context(tc.tile_pool(name="rc", bufs=4))
        ypool = ctx.enter_context(tc.tile_pool(name="ypool", bufs=2))
        stage_pool = ctx.enter_context(tc.tile_pool(name="stage", bufs=8))

        # ---- constants ----
        identity = singles.tile([128, 128], F32)
        make_identity(nc, identity)
        bias_bc = singles.tile([128, E], F32)
        ones_row = singles.tile([1, 64], BF16)
        nc.gpsimd.memset(ones_row.bitcast(U32), 0x3F803F80)

        wq_pool = tc.alloc_tile_pool(name="wq_pool", bufs=1)
        x_pool = tc.alloc_tile_pool(name="x_pool", bufs=1)

        wq = [wq_pool.tile([128, F], BF16, name=f"wqkv_{ei}")
              for ei in range(NE)]

        def dma_cast_w_one(ei, c0, c1, engine=None, pool=None, tag="stage",
                           bufs=8):
            cn = c1 - c0
            assert cn <= 384
            eng = engine or nc.sync
            st_t = (pool or stage_pool).tile([128, 384], F32, tag=tag,
                                             bufs=bufs, name=f"wst_{ei}_{c0}")
            eng.dma_start(
                out=st_t[:, 0:cn],
                in_=wqkv_d[ei * 128:(ei + 1) * 128, c0:c1])
            nc.gpsimd.tensor_copy(wq[ei][:, c0:c1], st_t[:, 0:cn])

        def dma_cast_w_cols(c0, c1):
            for ei in range(NE):
                dma_cast_w_one(ei, c0, c1)

        # ---- DMA priority order: x[0:4], QK0, x[4:8], V, QK123, QK45 ----
        xT = [wq_pool.tile([128, S], BF16, name=f"xT_{ei}")
              for ei in range(NE)]
        x_sb_all = []
        for st in range(NST):
            x_t = x_pool.tile([128, E], F32, tag="x", bufs=8, name=f"x_{st}")
            x_sb_all.append(x_t)
        # v_pad memsets first: gpsimd is idle until the first weight DMA lands
        v_pad = [sb.tile([128, H * DP], BF16, name=f"vpad_{st}")
                 for st in range(NST)]
        for st in range(NST):
            nc.gpsimd.memset(v_pad[st].bitcast(U32), 0x3F803F80)

        for k in range(4):
            nc.sync.dma_start(out=x_sb_all[k], in_=x_d[k * 128:(k + 1) * 128, :])
        dma_cast_w_cols(0, 128)                # Q pair 0
        dma_cast_w_cols(E, E + 128)            # K pair 0
        for k in range(4, 8):
            nc.sync.dma_start(out=x_sb_all[k], in_=x_d[k * 128:(k + 1) * 128, :])

        nc.sync.dma_start(
            out=bias_bc,
            in_=bass.AP(tensor=bout_d.tensor, offset=bout_d.offset,
                        ap=[[0, 128]] + list(bout_d.ap)))

        # Remaining weight streams (V / QK rest / w_out) are emitted inside
        # the slot schedule so queue program order matches arrival priority.
        # V rides the scalar engine's DMA queue with a dedicated staging
        # pool carved from x_pool's freed region, so its descriptors never
        # wait and never head-of-line block the exp stream.
        late_pools = {}
        wo = []
        y_sb = []

        def emit_vstage_pool():
            vpool = tc.alloc_tile_pool(name="vstage", bufs=1)
            late_pools["vstage"] = vpool

        def emit_v_dma(i):
            blk = (2 * E, 2 * E + 384) if i < NE else (2 * E + 384, F)
            dma_cast_w_one(i % NE, blk[0], blk[1], engine=nc.scalar,
                           pool=late_pools["vstage"], tag="vst", bufs=12)

        def emit_wo_ysb():
            # created after x_pool releases so they reuse its SBUF region
            wpool = tc.alloc_tile_pool(name="wpool", bufs=1)
            ysb_pool2 = tc.alloc_tile_pool(name="ysb2", bufs=1)
            late_pools["wpool"] = wpool
            late_pools["ysb2"] = ysb_pool2
            for st in range(NST):
                y_sb.append(ysb_pool2.tile([128, E], F32, name=f"ysb_{st}"))

        def emit_wo_dma(ei):
            wo_st = stage_pool.tile([128, E], F32, tag="wostage", bufs=3,
                                    name=f"wost_{ei}")
            nc.sync.dma_start(out=wo_st,
                              in_=wout_d[ei * 128:(ei + 1) * 128, :])
            wo_t = late_pools["wpool"].tile([128, E], BF16, name=f"wout_{ei}")
            nc.gpsimd.tensor_copy(wo_t, wo_st)
            wo.append(wo_t)

        def emit_transpose_half(half):
            for ei in range(NE):
                ps_xt = ps.tile([128, 512], F32, tag="mm", bufs=2,
                                name=f"psxt_{ei}_{half}")
                for k in range(4):
                    nc.tensor.transpose(
                        ps_xt[:, k * 128:(k + 1) * 128],
                        x_sb_all[half * 4 + k][:, ei * 128:(ei + 1) * 128],
                        identity)
                dst = xT[ei][:, half * 512:(half + 1) * 512]
                if ei % 2 == 0:
                    nc.vector.tensor_copy(dst, ps_xt)
                else:
                    nc.scalar.copy(dst, ps_xt)

        emit_transpose_half(0)

        # ---- projection chunk emitters (PE fillers) ----
        qkT = [sb.tile([128, S], BF16, name=f"qkT_{ft}")
               for ft in range(2 * NE)]

        def emit_v_chunk(st, c0, cn):
            ps_v = ps.tile([128, 512], F32, tag="mm", bufs=2,
                           name=f"psv_{st}_{c0}")
            for ei in range(NE):
                nc.tensor.matmul(
                    ps_v[:, 0:cn],
                    xT[ei][:, st * 128:(st + 1) * 128],
                    wq[ei][:, 2 * E + c0:2 * E + c0 + cn],
                    start=(ei == 0), stop=(ei == NE - 1))
            vp3 = v_pad[st].rearrange("p (h c) -> p h c", c=DP)
            h0 = c0 // D
            nc.vector.tensor_copy(
                vp3[:, h0:h0 + cn // D, 0:D],
                ps_v[:, 0:cn].rearrange("p (h d) -> p h d", d=D))

        def emit_qkt_chunk(ft, sc, on_act=False):
            ps_q = ps.tile([128, 512], F32, tag="mm", bufs=2,
                           name=f"psq_{ft}_{sc}")
            for ei in range(NE):
                nc.tensor.matmul(
                    ps_q,
                    wq[ei][:, ft * 128:(ft + 1) * 128],
                    xT[ei][:, sc * 512:(sc + 1) * 512],
                    start=(ei == 0), stop=(ei == NE - 1))
            dst = qkT[ft][:, sc * 512:(sc + 1) * 512]
            if on_act:
                nc.scalar.copy(dst, ps_q)
            else:
                nc.vector.tensor_copy(dst, ps_q)

        def sched_qkt_split(g, ft, sc):
            # halves straddle the slot boundary: half A is the last PE work
            # of slot g, half B the first of slot g+1, sharing one psum tile
            # (safe: no other mm-tag alloc can land in between)
            state = {}

            def half_a():
                ps_q = ps.tile([128, 512], F32, tag="mm", bufs=2,
                               name=f"psq_{ft}_{sc}")
                state["ps"] = ps_q
                for ei in range(3):
                    nc.tensor.matmul(
                        ps_q,
                        wq[ei][:, ft * 128:(ft + 1) * 128],
                        xT[ei][:, sc * 512:(sc + 1) * 512],
                        start=(ei == 0), stop=False)

            def half_b():
                ps_q = state["ps"]
                for ei in range(3, NE):
                    nc.tensor.matmul(
                        ps_q,
                        wq[ei][:, ft * 128:(ft + 1) * 128],
                        xT[ei][:, sc * 512:(sc + 1) * 512],
                        start=False, stop=(ei == NE - 1))
                nc.vector.tensor_copy(
                    qkT[ft][:, sc * 512:(sc + 1) * 512], ps_q)

            at(g, half_a)
            at_front(g + 1, half_b)

        # prelude: pair-0 Q/K lower halves gate the first scores
        emit_qkt_chunk(0, 0, on_act=True)
        emit_qkt_chunk(NE, 0, on_act=True)

        # ---- output projection partials ----
        attnT = [sb.tile([128, S], BF16, name=f"attnT_{j}")
                 for j in range(NPAIR)]

        def emit_ypartial_a(st, c0, cn):
            # ej = 0..3 partial contraction + bias, into SBUF
            ps_y = ps.tile([128, 512], F32, tag="mm", bufs=2,
                           name=f"psyA_{st}_{c0}")
            for ej in range(4):
                nc.tensor.matmul(
                    ps_y[:, 0:cn],
                    attnT[ej][:, st * 128:(st + 1) * 128],
                    wo[ej][:, c0:c0 + cn],
                    start=(ej == 0), stop=(ej == 3))
            nc.vector.tensor_add(y_sb[st][:, c0:c0 + cn], ps_y[:, 0:cn],
                                 bias_bc[:, c0:c0 + cn])

        def emit_ypartial_b(st):
            # ej = 4 accumulated into y_sb (runs in early drain, gated only
            # on attnT[4])
            for (c0, cn) in ((0, 512), (512, 256)):
                ps_y = ps.tile([128, 512], F32, tag="mm", bufs=2,
                               name=f"psyB_{st}_{c0}")
                nc.tensor.matmul(
                    ps_y[:, 0:cn],
                    attnT[4][:, st * 128:(st + 1) * 128],
                    wo[4][:, c0:c0 + cn],
                    start=True, stop=True)
                nc.vector.tensor_add(y_sb[st][:, c0:c0 + cn],
                                     y_sb[st][:, c0:c0 + cn],
                                     ps_y[:, 0:cn])

        def emit_ytail(st):
            # ej = 5 + partial + DMA out
            y_t = ypool.tile([128, E], F32, tag="y", name=f"y_{st}")
            for (c0, cn) in ((0, 512), (512, 256)):
                ps_y = ps.tile([128, 512], F32, tag="mm", bufs=2,
                               name=f"psyC_{st}_{c0}")
                nc.tensor.matmul(
                    ps_y[:, 0:cn],
                    attnT[5][:, st * 128:(st + 1) * 128],
                    wo[5][:, c0:c0 + cn],
                    start=True, stop=True)
                nc.vector.tensor_add(y_t[:, c0:c0 + cn], ps_y[:, 0:cn],
                                     y_sb[st][:, c0:c0 + cn])
            nc.sync.dma_start(out=y_d[st * 128:(st + 1) * 128, :], in_=y_t)

        # ---- slot schedule ----
        sched = {}

        def at(g, thunk):
            sched.setdefault(g, []).append(thunk)

        def at_front(g, thunk):
            sched.setdefault(g, []).insert(0, thunk)

        at(1, lambda: emit_transpose_half(1))
        at(2, emit_vstage_pool)
        for i in range(2 * NE):
            at(2 + i // 2, lambda i=i: emit_v_dma(i))
        at(3, lambda: emit_qkt_chunk(NE, 1))
        at(4, emit_wo_ysb)
        at(5, lambda: emit_qkt_chunk(0, 1))
        # QK pairs 1-3 / 4-5 weight DMA+cast, paced 2-3 per slot on the sync
        # queue; w_out trails them
        qk_blocks = [(ei, 128, 512) for ei in range(NE)] + \
                    [(ei, E + 128, E + 512) for ei in range(NE)]
        for i, (ei, c0, c1) in enumerate(qk_blocks):
            at(8 + i // 2, lambda ei=ei, c0=c0, c1=c1:
               dma_cast_w_one(ei, c0, c1))
        qk_blocks2 = [(ei, 512, E) for ei in range(NE)] + \
                     [(ei, E + 512, 2 * E) for ei in range(NE)]
        for i, (ei, c0, c1) in enumerate(qk_blocks2):
            at(14 + i // 3, lambda ei=ei, c0=c0, c1=c1:
               dma_cast_w_one(ei, c0, c1))
        for ei in range(NE):
            at(18 + ei, lambda ei=ei: emit_wo_dma(ei))
        # V chunk halves spread one per slot, finishing just before their
        # first PV consumer (pv(0,st) at slot st+8)
        for st in range(NST):
            at(7 + st, lambda st=st: emit_v_chunk(st, 0, 512))
            at(8 + st, lambda st=st: emit_v_chunk(st, 512, 256))
        # QK pairs 1-5: JIT with DMA arrival; chunk halves straddle slot
        # boundaries so no single slot carries a full 6-matmul chunk
        qkt_slots = {1: (12, 13, 14, 15), 2: (26, 28, 29, 30),
                     3: (42, 44, 45, 46), 4: (58, 60, 61, 62),
                     5: (66, 68, 69, 70)}
        for j in range(1, NPAIR):
            for off, (ft, sc) in zip(qkt_slots[j], ((j, 0), (NE + j, 0),
                                                    (j, 1), (NE + j, 1))):
                sched_qkt_split(off, ft, sc)
        # output-projection partials (ej 0..3) as late-phase fillers, one
        # (st, chunk) piece per slot; attnT[3] is ready at slot 72
        for i in range(NST * 2):
            st, ci = divmod(i, 2)
            c0, cn = ((0, 512), (512, 256))[ci]
            at(73 + i * 23 // 16,
               lambda st=st, c0=c0, cn=cn: emit_ypartial_a(st, c0, cn))


        # ---- attention state ----
        expst_live = {}     # (p, kc) -> expst tile
        pspv_live = {}      # p -> [ps_pv hh0, hh1]
        pvsb_live = {}      # p -> [pv_sb hh0, hh1]
        rcs_live = {}       # p -> [rc hh0, hh1]

        def emit_scores_exp(p, kc):
            j, qh = divmod(p, 2)
            q0 = qh * 512
            expst = expst_pool.tile([128, 1024], BF16, tag="expst",
                                    name=f"expst_{p}_{kc}")
            expst_live[(p, kc)] = expst
            ps_s = ps.tile([128, 1024], F32, tag="scores", bufs=2,
                           name=f"pss_{p}_{kc}")
            for hh in range(2):
                nc.tensor.matmul(
                    ps_s[:, hh * 512:(hh + 1) * 512],
                    qkT[NE + j][hh * 64:(hh + 1) * 64,
                                kc * 128:(kc + 1) * 128],
                    qkT[j][hh * 64:(hh + 1) * 64, q0:q0 + 512],
                    start=True, stop=True,
                    tile_position=(hh * 64, 0))
            nc.scalar.activation(
                out=expst, in_=ps_s,
                func=mybir.ActivationFunctionType.Exp,
                scale=0.125)

        def emit_pv(p, kc):
            j = p // 2
            if kc == 0:
                pspv_live[p] = [ps.tile([DP, 512], F32, tag="pv", bufs=2,
                                        name=f"pspv_{p}_{hh}")
                                for hh in range(2)]
            ps_pv = pspv_live[p]
            expst = expst_live.pop((p, kc))
            for hh in range(2):
                nc.tensor.matmul(
                    ps_pv[hh],
                    v_pad[kc][:, (2 * j + hh) * DP:(2 * j + hh + 1) * DP],
                    expst[:, hh * 512:(hh + 1) * 512],
                    start=(kc == 0), stop=(kc == NKC - 1))

        def emit_finish(p):
            # drain psum to SBUF right away: the pv banks hand off to phase
            # p+1 with a full slot of slack, normalize runs off-path
            ps_pv = pspv_live.pop(p)
            rcs = []
            pv_t = pvsb_pool.tile([128, 512], F32, tag="pvsb",
                                  name=f"pvsb_{p}")
            for hh in range(2):
                rc_t = rc_pool.tile([1, 512], BF16, tag="rc",
                                    name=f"rc_{p}_{hh}")
                nc.vector.tensor_copy(rc_t, ps_pv[hh][D:DP, :])
                rcs.append(rc_t)
                nc.vector.tensor_copy(pv_t[hh * 64:(hh + 1) * 64, :],
                                      ps_pv[hh][0:D, :])
            rcs_live[p] = rcs
            pvsb_live[p] = pv_t

        def emit_norm(p):
            j, qh = divmod(p, 2)
            q0 = qh * 512
            rcs = rcs_live.pop(p)
            pv_t = pvsb_live.pop(p)
            bc_ps = ps.tile([128, 512], F32, tag="mm", bufs=2,
                            name=f"bcps_{p}")
            for hh in range(2):
                nc.tensor.matmul(bc_ps[hh * 64:(hh + 1) * 64, :],
                                 ones_row, rcs[hh],
                                 start=True, stop=True,
                                 tile_position=(0, hh * 64))
            bc_sb = bcast_pool.tile([128, 512], F32, tag="bc",
                                    name=f"bc_{p}")
            nc.vector.reciprocal_approx_fast(out=bc_sb, in_=bc_ps)
            nc.vector.tensor_mul(attnT[j][:, q0:q0 + 512], pv_t, bc_sb)

        # pv(p, kc) at slot p*8+kc+PVLAG; finish right after pv(p,7) in the
        # same slot (frees the pv psum banks), norm one slot later.
        for p in range(NPH):
            for kc in range(NKC):
                at(p * NKC + kc + PVLAG, lambda p=p, kc=kc: emit_pv(p, kc))
            at(p * NKC + (NKC - 1) + PVLAG, lambda p=p: emit_finish(p))
            at(p * NKC + NKC + PVLAG, lambda p=p: emit_norm(p))

        # ej=4 partials ride the early drain (attnT[4] ready at slot 88),
        # the ej=5 tail pieces follow attnT[5] (ready at slot 104); added
        # after the pv/norm thunks so they never delay the final normalize
        for st in range(NST):
            at(97 + st, lambda st=st: emit_ypartial_b(st))
        for st in range(NST):
            at(105 + st, lambda st=st: emit_ytail(st))

        # ---- run the slot schedule ----
        for g in range(NSLOT + NDRAIN):
            if g < NSLOT:
                p, kc = divmod(g, NKC)
                emit_scores_exp(p, kc)
            if g == 2:
                x_pool.release()
            for thunk in sched.pop(g, ()):
                thunk()
        assert not sched, sorted(sched)
        late_pools["ysb2"].release()
        late_pools["wpool"].release()
        late_pools["vstage"].release()
        wq_pool.release()


_NC_CACHE = None


def _get_nc():
    global _NC_CACHE
    if _NC_CACHE is None:
        _NC_CACHE = _build()
    return _NC_CACHE


def kernel(x, w_qkv, w_out, b_out, _trace=False, **_run_kwargs):
    """Full-input MHA: x [8,1024,768] f32 -> y [8,1024,768] f32."""
    nc = _get_nc()
    x = np.ascontiguousarray(np.asarray(x, dtype=np.float32))
    w_qkv = np.ascontiguousarray(np.asarray(w_qkv, dtype=np.float32))
    w_out = np.ascontiguousarray(np.asarray(w_out, dtype=np.float32))
    b_out = np.ascontiguousarray(np.asarray(b_out, dtype=np.float32))
    in_maps = [
        {"x": x[i], "w_qkv": w_qkv, "w_out": w_out, "b_out": b_out}
        for i in range(NCORES)
    ]
    res = run_bass_kernel_spmd(nc, in_maps, core_ids=list(range(NCORES)),
                               trace=_trace, **_run_kwargs)
    y = np.stack([res.results[i]["y"] for i in range(NCORES)], axis=0)
    if _trace:
        return y, res
    return y


# revision 48
# speedup vs baseline: 1.1521x; 1.1521x over previous
"""Multi-head attention (B=8, S=1024, E=768, H=12, D=64) on 8 TRN2 NeuronCores.

Sharding: data-parallel over batch. Core i computes batch element i end to end;
weights are replicated. No collectives.

Per-core dataflow (matmuls in bf16; f32 PSUM accumulation):
  1. x [S,E] -> PE-transpose -> xT [E,S] (bf16)
  2. w_qkv DMA'd f32 into staging, cast to bf16 on GPSIMD (idle engine)
  3. QT/KT = w_qkv.T @ xT; V -> V_pad [S, H*(D+1)] bf16 with a ones column
  4. attention as a flat schedule of 96 slots (12 head-pair-phases x 8 key
     chunks). Slot g: scores (2 concurrent K=64 matmuls via tile_position)
     + one [128,1024] exp on ACT; PV matmuls run LAGGED by a full phase (8
     slots) so the PE never head-of-line blocks on late V DMA and the PV
     psum hand-off carries no bubble. Each phase's PV result is copied to
     SBUF immediately (freeing the psum bank), then normalized off the
     critical path (denominator broadcast matmul + fast reciprocal + mul).
  5. y = attnT.T @ w_out + b_out (bf16); the ej=0..3 contraction chunks run
     as fillers inside late attention slots, only ej=4..5 trail the loop.

Projection work (QKT/V chunks) is scheduled into specific slots as PE
filler, just-in-time with DMA arrival: x, QK pair 0, V, QK pairs 1-3, 4-5.

PSUM budget (8 banks): scores 2 x 2 + pv 2 x 1 + mm 2 x 1.
"""

import numpy as np

import concourse.bass as bass
import concourse.bacc as bacc
import concourse.tile as tile
from concourse import mybir
from concourse.bass_utils import run_bass_kernel_spmd
from concourse.bass_interp import get_hw_module
from concourse.masks import make_identity

F32 = mybir.dt.float32
F32R = mybir.dt.float32r
BF16 = mybir.dt.bfloat16
U32 = mybir.dt.uint32

B, S, E = 8, 1024, 768
H, D = 12, 64
F = 3 * E                  # 2304
NCORES = 8
NPAIR = H // 2             # 6 head pairs
NKC = S // 128             # 8 key chunks
NST = S // 128             # 8 sequence tiles
NE = E // 128              # 6 embedding chunks
DP = D + 1                 # 65: head dim + ones column
NPH = 2 * NPAIR            # 12 phases (pair, q-half)
PVLAG = 8                  # PV trails scores by one full phase
NSLOT = NPH * NKC          # 96 scored slots
NDRAIN = 20                # trailing slots to flush pv + norm tails


def _build():
    nc = bacc.Bacc("TRN2", target_bir_lowering=False, debug=False,
                   num_devices=NCORES)

    x_d = nc.dram_tensor("x", [S, E], F32, kind="ExternalInput").ap()
    wqkv_d = nc.dram_tensor("w_qkv", [E, F], F32, kind="ExternalInput").ap()
    wout_d = nc.dram_tensor("w_out", [E, E], F32, kind="ExternalInput").ap()
    bout_d = nc.dram_tensor("b_out", [E], F32, kind="ExternalInput").ap()
    y_d = nc.dram_tensor("y", [S, E], F32, kind="ExternalOutput").ap()

    with tile.TileContext(nc) as tc:
        _emit(nc, tc, x_d, wqkv_d, wout_d, bout_d, y_d)

    nc.compile()
    nc.m = get_hw_module(nc.m)
    return nc


def _emit(nc, tc, x_d, wqkv_d, wout_d, bout_d, y_d):
    from contextlib import ExitStack
    ctx = ExitStack()
    with ctx:
        singles = ctx.enter_context(tc.tile_pool(name="singles", bufs=1))
        sb = ctx.enter_context(tc.tile_pool(name="sb", bufs=1))
        ps = ctx.enter_context(tc.tile_pool(name="ps", bufs=1, space="PSUM"))
        expst_pool = ctx.enter_context(tc.tile_pool(name="expst", bufs=10))
        bcast_pool = ctx.enter_context(tc.tile_pool(name="bcast", bufs=2))
        pvsb_pool = ctx.enter_context(tc.tile_pool(name="pvsb", bufs=4))
        rc_pool = ctx.enter_context(tc.tile_pool(name="rc", bufs=4))
        ypool = ctx.enter_context(tc.tile_pool(name="ypool", bufs=2))
        stage_pool = ctx.enter_context(tc.tile_pool(name="stage", bufs=8))

        # ---- constants ----
        identity = singles.tile([128, 128], F32)
        make_identity(nc, identity)
        bias_bc = singles.tile([128, E], F32)
        ones_row = singles.tile([1, 64], BF16)
        nc.gpsimd.memset(ones_row.bitcast(U32), 0x3F803F80)

        wq_pool = tc.alloc_tile_pool(name="wq_pool", bufs=1)
        x_pool = tc.alloc_tile_pool(name="x_pool", bufs=1)

        wq = [wq_pool.tile([128, F], BF16, name=f"wqkv_{ei}")
              for ei in range(NE)]

        def dma_cast_w_one(ei, c0, c1, engine=None, pool=None, tag="stage",
                           bufs=8):
            cn = c1 - c0
            assert cn <= 384
            eng = engine or nc.sync
            st_t = (pool or stage_pool).tile([128, 384], F32, tag=tag,
                                             bufs=bufs, name=f"wst_{ei}_{c0}")
            eng.dma_start(
                out=st_t[:, 0:cn],
                in_=wqkv_d[ei * 128:(ei + 1) * 128, c0:c1])
            nc.gpsimd.tensor_copy(wq[ei][:, c0:c1], st_t[:, 0:cn])

        def dma_cast_w_cols(c0, c1):
            for ei in range(NE):
                dma_cast_w_one(ei, c0, c1)

        # ---- DMA priority order: x[0:4], QK0, x[4:8], V, QK123, QK45 ----
        xT = [wq_pool.tile([128, S], BF16, name=f"xT_{ei}")
              for ei in range(NE)]
        x_sb_all = []
        for st in range(NST):
            x_t = x_pool.tile([128, E], F32, tag="x", bufs=8, name=f"x_{st}")
            x_sb_all.append(x_t)
        # v_pad memsets first: gpsimd is idle until the first weight DMA lands
        v_pad = [sb.tile([128, H * DP], BF16, name=f"vpad_{st}")
                 for st in range(NST)]
        for st in range(NST):
            nc.gpsimd.memset(v_pad[st].bitcast(U32), 0x3F803F80)

        for k in range(4):
            nc.sync.dma_start(out=x_sb_all[k], in_=x_d[k * 128:(k + 1) * 128, :])
        dma_cast_w_cols(0, 128)                # Q pair 0
        dma_cast_w_cols(E, E + 128)            # K pair 0
        for k in range(4, 8):
            nc.sync.dma_start(out=x_sb_all[k], in_=x_d[k * 128:(k + 1) * 128, :])

        dma_cast_w_cols(2 * E, 2 * E + 384)    # V first half
        dma_cast_w_cols(2 * E + 384, F)        # V second half
        dma_cast_w_cols(128, 512)              # Q pairs 1-3
        dma_cast_w_cols(E + 128, E + 512)      # K pairs 1-3
        dma_cast_w_cols(512, E)                # Q pairs 4-5
        dma_cast_w_cols(E + 512, 2 * E)        # K pairs 4-5

        # w_out + bias: DMA now (behind all wq blocks in the sync queue);
        # the bf16 cast targets are created later, after x_pool releases.
        wo_stages = []
        for ei in range(NE):
            wo_st = stage_pool.tile([128, E], F32, tag="wostage", bufs=3,
                                    name=f"wost_{ei}")
            nc.sync.dma_start(out=wo_st,
                              in_=wout_d[ei * 128:(ei + 1) * 128, :])
            wo_stages.append(wo_st)
        nc.sync.dma_start(
            out=bias_bc,
            in_=bass.AP(tensor=bout_d.tensor, offset=bout_d.offset,
                        ap=[[0, 128]] + list(bout_d.ap)))
        late_pools = {}
        wo = []
        y_sb = []

        def emit_wo_ysb():
            # created after x_pool releases so they reuse its SBUF region
            wpool = tc.alloc_tile_pool(name="wpool", bufs=1)
            ysb_pool2 = tc.alloc_tile_pool(name="ysb2", bufs=1)
            late_pools["wpool"] = wpool
            late_pools["ysb2"] = ysb_pool2
            for ei in range(NE):
                wo_t = wpool.tile([128, E], BF16, name=f"wout_{ei}")
                nc.gpsimd.tensor_copy(wo_t, wo_stages[ei])
                wo.append(wo_t)
            for st in range(NST):
                y_sb.append(ysb_pool2.tile([128, E], F32, name=f"ysb_{st}"))

        def emit_transpose_half(half):
            for ei in range(NE):
                ps_xt = ps.tile([128, 512], F32, tag="mm", bufs=2,
                                name=f"psxt_{ei}_{half}")
                for k in range(4):
                    nc.tensor.transpose(
                        ps_xt[:, k * 128:(k + 1) * 128],
                        x_sb_all[half * 4 + k][:, ei * 128:(ei + 1) * 128],
                        identity)
                dst = xT[ei][:, half * 512:(half + 1) * 512]
                if ei % 2 == 0:
                    nc.vector.tensor_copy(dst, ps_xt)
                else:
                    nc.scalar.copy(dst, ps_xt)

        emit_transpose_half(0)

        # ---- projection chunk emitters (PE fillers) ----
        qkT = [sb.tile([128, S], BF16, name=f"qkT_{ft}")
               for ft in range(2 * NE)]

        def emit_v_chunk(st, c0, cn):
            ps_v = ps.tile([128, 512], F32, tag="mm", bufs=2,
                           name=f"psv_{st}_{c0}")
            for ei in range(NE):
                nc.tensor.matmul(
                    ps_v[:, 0:cn],
                    xT[ei][:, st * 128:(st + 1) * 128],
                    wq[ei][:, 2 * E + c0:2 * E + c0 + cn],
                    start=(ei == 0), stop=(ei == NE - 1))
            vp3 = v_pad[st].rearrange("p (h c) -> p h c", c=DP)
            h0 = c0 // D
            nc.vector.tensor_copy(
                vp3[:, h0:h0 + cn // D, 0:D],
                ps_v[:, 0:cn].rearrange("p (h d) -> p h d", d=D))

        def emit_qkt_chunk(ft, sc, on_act=False):
            ps_q = ps.tile([128, 512], F32, tag="mm", bufs=2,
                           name=f"psq_{ft}_{sc}")
            for ei in range(NE):
                nc.tensor.matmul(
                    ps_q,
                    wq[ei][:, ft * 128:(ft + 1) * 128],
                    xT[ei][:, sc * 512:(sc + 1) * 512],
                    start=(ei == 0), stop=(ei == NE - 1))
            dst = qkT[ft][:, sc * 512:(sc + 1) * 512]
            if on_act:
                nc.scalar.copy(dst, ps_q)
            else:
                nc.vector.tensor_copy(dst, ps_q)

        def sched_qkt_split(g, ft, sc):
            # halves straddle the slot boundary: half A is the last PE work
            # of slot g, half B the first of slot g+1, sharing one psum tile
            # (safe: no other mm-tag alloc can land in between)
            state = {}

            def half_a():
                ps_q = ps.tile([128, 512], F32, tag="mm", bufs=2,
                               name=f"psq_{ft}_{sc}")
                state["ps"] = ps_q
                for ei in range(3):
                    nc.tensor.matmul(
                        ps_q,
                        wq[ei][:, ft * 128:(ft + 1) * 128],
                        xT[ei][:, sc * 512:(sc + 1) * 512],
                        start=(ei == 0), stop=False)

            def half_b():
                ps_q = state["ps"]
                for ei in range(3, NE):
                    nc.tensor.matmul(
                        ps_q,
                        wq[ei][:, ft * 128:(ft + 1) * 128],
                        xT[ei][:, sc * 512:(sc + 1) * 512],
                        start=False, stop=(ei == NE - 1))
                nc.vector.tensor_copy(
                    qkT[ft][:, sc * 512:(sc + 1) * 512], ps_q)

            at(g, half_a)
            at_front(g + 1, half_b)

        # prelude: pair-0 Q/K lower halves gate the first scores
        emit_qkt_chunk(0, 0, on_act=True)
        emit_qkt_chunk(NE, 0, on_act=True)

        # ---- output projection partials ----
        attnT = [sb.tile([128, S], BF16, name=f"attnT_{j}")
                 for j in range(NPAIR)]

        def emit_ypartial_a(st, c0, cn):
            # ej = 0..3 partial contraction + bias, into SBUF
            ps_y = ps.tile([128, 512], F32, tag="mm", bufs=2,
                           name=f"psyA_{st}_{c0}")
            for ej in range(4):
                nc.tensor.matmul(
                    ps_y[:, 0:cn],
                    attnT[ej][:, st * 128:(st + 1) * 128],
                    wo[ej][:, c0:c0 + cn],
                    start=(ej == 0), stop=(ej == 3))
            nc.vector.tensor_add(y_sb[st][:, c0:c0 + cn], ps_y[:, 0:cn],
                                 bias_bc[:, c0:c0 + cn])

        def emit_ypartial_b(st):
            # ej = 4 accumulated into y_sb (runs in early drain, gated only
            # on attnT[4])
            for (c0, cn) in ((0, 512), (512, 256)):
                ps_y = ps.tile([128, 512], F32, tag="mm", bufs=2,
                               name=f"psyB_{st}_{c0}")
                nc.tensor.matmul(
                    ps_y[:, 0:cn],
                    attnT[4][:, st * 128:(st + 1) * 128],
                    wo[4][:, c0:c0 + cn],
                    start=True, stop=True)
                nc.vector.tensor_add(y_sb[st][:, c0:c0 + cn],
                                     y_sb[st][:, c0:c0 + cn],
                                     ps_y[:, 0:cn])

        def emit_ytail(st):
            # ej = 5 + partial + DMA out
            y_t = ypool.tile([128, E], F32, tag="y", name=f"y_{st}")
            for (c0, cn) in ((0, 512), (512, 256)):
                ps_y = ps.tile([128, 512], F32, tag="mm", bufs=2,
                               name=f"psyC_{st}_{c0}")
                nc.tensor.matmul(
                    ps_y[:, 0:cn],
                    attnT[5][:, st * 128:(st + 1) * 128],
                    wo[5][:, c0:c0 + cn],
                    start=True, stop=True)
                nc.vector.tensor_add(y_t[:, c0:c0 + cn], ps_y[:, 0:cn],
                                     y_sb[st][:, c0:c0 + cn])
            nc.sync.dma_start(out=y_d[st * 128:(st + 1) * 128, :], in_=y_t)

        # ---- slot schedule ----
        sched = {}

        def at(g, thunk):
            sched.setdefault(g, []).append(thunk)

        def at_front(g, thunk):
            sched.setdefault(g, []).insert(0, thunk)

        at(1, lambda: emit_transpose_half(1))
        at(3, lambda: emit_qkt_chunk(NE, 1))
        at(4, emit_wo_ysb)
        at(5, lambda: emit_qkt_chunk(0, 1))
        # V chunk halves spread one per slot, finishing just before their
        # first PV consumer (pv(0,st) at slot st+8)
        for st in range(NST):
            at(7 + st, lambda st=st: emit_v_chunk(st, 0, 512))
            at(8 + st, lambda st=st: emit_v_chunk(st, 512, 256))
        # QK pairs 1-5: JIT with DMA arrival; chunk halves straddle slot
        # boundaries so no single slot carries a full 6-matmul chunk
        qkt_slots = {1: (12, 13, 14, 15), 2: (26, 28, 29, 30),
                     3: (42, 44, 45, 46), 4: (58, 60, 61, 62),
                     5: (66, 68, 69, 70)}
        for j in range(1, NPAIR):
            for off, (ft, sc) in zip(qkt_slots[j], ((j, 0), (NE + j, 0),
                                                    (j, 1), (NE + j, 1))):
                sched_qkt_split(off, ft, sc)
        # output-projection partials (ej 0..3) as late-phase fillers, one
        # (st, chunk) piece per slot; attnT[3] is ready at slot 72
        for i in range(NST * 2):
            st, ci = divmod(i, 2)
            c0, cn = ((0, 512), (512, 256))[ci]
            at(73 + i * 23 // 16,
               lambda st=st, c0=c0, cn=cn: emit_ypartial_a(st, c0, cn))


        # ---- attention state ----
        expst_live = {}     # (p, kc) -> expst tile
        pspv_live = {}      # p -> [ps_pv hh0, hh1]
        pvsb_live = {}      # p -> [pv_sb hh0, hh1]
        rcs_live = {}       # p -> [rc hh0, hh1]

        def emit_scores_exp(p, kc):
            j, qh = divmod(p, 2)
            q0 = qh * 512
            expst = expst_pool.tile([128, 1024], BF16, tag="expst",
                                    name=f"expst_{p}_{kc}")
            expst_live[(p, kc)] = expst
            ps_s = ps.tile([128, 1024], F32, tag="scores", bufs=2,
                           name=f"pss_{p}_{kc}")
            for hh in range(2):
                nc.tensor.matmul(
                    ps_s[:, hh * 512:(hh + 1) * 512],
                    qkT[NE + j][hh * 64:(hh + 1) * 64,
                                kc * 128:(kc + 1) * 128],
                    qkT[j][hh * 64:(hh + 1) * 64, q0:q0 + 512],
                    start=True, stop=True,
                    tile_position=(hh * 64, 0))
            nc.scalar.activation(
                out=expst, in_=ps_s,
                func=mybir.ActivationFunctionType.Exp,
                scale=0.125)

        def emit_pv(p, kc):
            j = p // 2
            if kc == 0:
                pspv_live[p] = [ps.tile([DP, 512], F32, tag="pv", bufs=2,
                                        name=f"pspv_{p}_{hh}")
                                for hh in range(2)]
            ps_pv = pspv_live[p]
            expst = expst_live.pop((p, kc))
            for hh in range(2):
                nc.tensor.matmul(
                    ps_pv[hh],
                    v_pad[kc][:, (2 * j + hh) * DP:(2 * j + hh + 1) * DP],
                    expst[:, hh * 512:(hh + 1) * 512],
                    start=(kc == 0), stop=(kc == NKC - 1))

        def emit_finish(p):
            # drain psum to SBUF right away: the pv banks hand off to phase
            # p+1 with a full slot of slack, normalize runs off-path
            ps_pv = pspv_live.pop(p)
            rcs = []
            pv_t = pvsb_pool.tile([128, 512], F32, tag="pvsb",
                                  name=f"pvsb_{p}")
            for hh in range(2):
                rc_t = rc_pool.tile([1, 512], BF16, tag="rc",
                                    name=f"rc_{p}_{hh}")
                nc.vector.tensor_copy(rc_t, ps_pv[hh][D:DP, :])
                rcs.append(rc_t)
                nc.vector.tensor_copy(pv_t[hh * 64:(hh + 1) * 64, :],
                                      ps_pv[hh][0:D, :])
            rcs_live[p] = rcs
            pvsb_live[p] = pv_t

        def emit_norm(p):
            j, qh = divmod(p, 2)
            q0 = qh * 512
            rcs = rcs_live.pop(p)
            pv_t = pvsb_live.pop(p)
            bc_ps = ps.tile([128, 512], F32, tag="mm", bufs=2,
                            name=f"bcps_{p}")
            for hh in range(2):
                nc.tensor.matmul(bc_ps[hh * 64:(hh + 1) * 64, :],
                                 ones_row, rcs[hh],
                                 start=True, stop=True,
                                 tile_position=(0, hh * 64))
            bc_sb = bcast_pool.tile([128, 512], F32, tag="bc",
                                    name=f"bc_{p}")
            nc.vector.reciprocal_approx_fast(out=bc_sb, in_=bc_ps)
            nc.vector.tensor_mul(attnT[j][:, q0:q0 + 512], pv_t, bc_sb)

        # pv(p, kc) at slot p*8+kc+PVLAG; finish right after pv(p,7) in the
        # same slot (frees the pv psum banks), norm one slot later.
        for p in range(NPH):
            for kc in range(NKC):
                at(p * NKC + kc + PVLAG, lambda p=p, kc=kc: emit_pv(p, kc))
            at(p * NKC + (NKC - 1) + PVLAG, lambda p=p: emit_finish(p))
            at(p * NKC + NKC + PVLAG, lambda p=p: emit_norm(p))

        # ej=4 partials ride the early drain (attnT[4] ready at slot 88),
        # the ej=5 tail pieces follow attnT[5] (ready at slot 104); added
        # after the pv/norm thunks so they never delay the final normalize
        for st in range(NST):
            at(97 + st, lambda st=st: emit_ypartial_b(st))
        for st in range(NST):
            at(105 + st, lambda st=st: emit_ytail(st))

        # ---- run the slot schedule ----
        for g in range(NSLOT + NDRAIN):
            if g < NSLOT:
                p, kc = divmod(g, NKC)
                emit_scores_exp(p, kc)
            if g == 2:
                x_pool.release()
            for thunk in sched.pop(g, ()):
                thunk()
        assert not sched, sorted(sched)
        late_pools["ysb2"].release()
        late_pools["wpool"].release()
        wq_pool.release()


_NC_CACHE = None


def _get_nc():
    global _NC_CACHE
    if _NC_CACHE is None:
        _NC_CACHE = _build()
    return _NC_CACHE


def kernel(x, w_qkv, w_out, b_out, _trace=False, **_run_kwargs):
    """Full-input MHA: x [8,1024,768] f32 -> y [8,1024,768] f32."""
    nc = _get_nc()
    x = np.ascontiguousarray(np.asarray(x, dtype=np.float32))
    w_qkv = np.ascontiguousarray(np.asarray(w_qkv, dtype=np.float32))
    w_out = np.ascontiguousarray(np.asarray(w_out, dtype=np.float32))
    b_out = np.ascontiguousarray(np.asarray(b_out, dtype=np.float32))
    in_maps = [
        {"x": x[i], "w_qkv": w_qkv, "w_out": w_out, "b_out": b_out}
        for i in range(NCORES)
    ]
    res = run_bass_kernel_spmd(nc, in_maps, core_ids=list(range(NCORES)),
                               trace=_trace, **_run_kwargs)
    y = np.stack([res.results[i]["y"] for i in range(NCORES)], axis=0)
    if _trace:
        return y, res
    return y
